# revision 4
# baseline (speedup 1.0000x reference)
"""Trainium2 Bass kernel for a BiQRNN3D layer.

reference math:
  gates = conv3d(x, W, SAME, 3x3x3) + b          x: [2,16,31,256,256] f32
  Z, F1, F2 = split(gates, 3, channel)           W: [48,16,3,3,3], b: [48]
  Z = tanh(Z); F1 = sigmoid(F1); F2 = sigmoid(F2)
  h_fwd: depth-forward  recurrence h = F1*h + (1-F1)*Z
  h_bwd: depth-backward recurrence h = F2*h + (1-F2)*Z
  out = h_fwd + h_bwd                            [2,16,31,256,256] f32

Distribution: H (=256) is sharded 32 rows per core across 8 NeuronCores
(SPMD, identical program; each core's x shard carries its 1-row conv halo
with global-edge zeros baked in by the host).

Per-core pipeline (v2 - no DRAM gates round-trip):
  * conv as matmul, K = (kd,ci) = 48 contraction rows. The moving x tile
    holds 3 kd-shifted copies in partitions 0-47 (block A) and an
    additional h+1-shifted copy in partitions 64-111 (block B,
    host-prepared). Partition 48 is a ones-row (bias rides as a stationary
    row); partitions 49-63 are zeros.
  * M = 96: stationary columns (j, co) produce BOTH output h rows of an
    h-block at once. Per psum tile [96, 2*256] six K=112 matmuls
    accumulate: passes (p in {0,1}) x (kw in {0,1,2}); pass p streams x
    rows at tile-h 2p, and blocks A/B provide taps kh = 2p-j and 2p+1-j.
  * gates evac psum -> SBUF f16 tile g[96, 32, 256] (d-major). Slot 31
    holds a "j-swap" copy of d=30 for j=1 (6 extra M=48 matmuls with
    j-swapped stationary columns), so both d=30 transposes below read
    from partition base 0.
  * on-chip transpose via PE is_transpose matmuls into PSUM f16 tile
    tp[128, 3 banks, 1024]: per w-half, 30x [96,128] transposes at slot
    (d//10, (d%10)*96) plus 2x [48,128] for d=30 into the 128-f16 bank
    gaps (offset 960). No matmul group crosses a 2KB psum bank.
  * ACT: tanh/sigmoid read strided from psum tp at 128-partition
    utilization; DVE: g = (f-1)*z, tensor_tensor_scan (h = f*h - g) for
    both directions (backward stored d-reversed); f zeroed at d=0 so one
    long scan chains safely across channel runs. out fp32 [S, 16, 31]
    -> host reassembles.
"""

from contextlib import ExitStack

import numpy as np

import concourse.bass as bass
import concourse.tile as tile
from concourse import bacc, mybir

F32 = mybir.dt.float32
F16 = mybir.dt.float16
AF = mybir.ActivationFunctionType
ALU = mybir.AluOpType

N_CORES = 8
B = 2
CIN = 16
HID = 16
CO = 3 * HID            # 48
D = 31
H = 256
W = 256
HSH = H // N_CORES      # 32
HB = 2                  # output h rows per conv tile (= M/CO)
DC = 2                  # d slices per psum tile
WP = W + 2
S = B * HSH * W         # 16384
FX = D * 2 * WP         # x tile free extent per partition
CHUNK = 128
NST = 6                 # stationary matrices (main)
WCOLS = NST * 2 * CO + NST * CO   # 576 + 288 = 864


def _build_program(reps=1, do_conv=True, do_scan=True, do_tp=True):
    nc = bacc.Bacc("TRN2", target_bir_lowering=False, debug=False)

    x_dram = nc.dram_tensor("x", [CIN, D + 2, B, HSH + 2, WP], F16,
                            kind="ExternalInput").ap()
    wts = nc.dram_tensor("wts", [128, WCOLS], F16,
                         kind="ExternalInput").ap()
    aux = nc.dram_tensor("aux", [16, FX], F16, kind="ExternalInput").ap()
    idn = nc.dram_tensor("idn", [96, 96], F16, kind="ExternalInput").ap()
    out = nc.dram_tensor("out", [S, HID, D], F32, kind="ExternalOutput").ap()

    with tile.TileContext(nc) as tc, ExitStack() as ctx:
        wsb = nc.alloc_sbuf_tensor("wsb", [128, WCOLS], F16).ap()
        idsb = nc.alloc_sbuf_tensor("idsb", [96, 96], F16).ap()
        # x tile: A rows hold x at h = h0 + 2t, B rows x at h0 + 1 + 2t
        xbufs = [nc.alloc_sbuf_tensor(f"xb{i}", [112, D, 2, WP], F16).ap()
                 for i in range(2)]

        nc.sync.dma_start(wsb, wts)
        nc.sync.dma_start(idsb, idn)
        for xb in xbufs:
            nc.sync.dma_start(
                xb[48:64].rearrange("p a b c -> p (a b c)"), aux)

        g_pool = ctx.enter_context(tc.tile_pool(name="gp", bufs=2))
        ps_pool = ctx.enter_context(tc.tile_pool(name="ps", bufs=2,
                                                 space="PSUM"))
        tp_pool = ctx.enter_context(tc.tile_pool(name="tp", bufs=2,
                                                 space="PSUM"))
        sc_pool = ctx.enter_context(tc.tile_pool(name="sc", bufs=4))

        n_hblk = HSH // HB
        n_dc = (D + DC - 1) // DC

        def scan_chunk(tp, j, c0):
            """One 128-pixel chunk: activations from psum tp, then scan."""
            zt = sc_pool.tile([128, HID, D], F16, tag="zt")
            f1 = sc_pool.tile([128, HID, D], F16, tag="f1")
            f2 = sc_pool.tile([128, HID, D], F16, tag="f2")
            # main d 0..29 at psum slot (d//10, (d%10)*96), cols j*48+c
            vm = tp[:, :, 0:960].rearrange("p b (db c) -> p c b db", c=96)
            f2r = f2[:, :, ::-1]  # f2r[:, :, d] == f2[:, :, 30-d]
            for gi, (dst, fn) in enumerate((
                    (zt, AF.Tanh), (f1, AF.Sigmoid), (f2r, AF.Sigmoid))):
                cb = j * CO + gi * HID
                nc.scalar.activation(
                    dst[:, :, 0:30].rearrange("p c (b db) -> p c b db", b=3),
                    vm[:, cb:cb + HID], fn)
                # d=30 lives in the bank-j gap at offset 960, cols 0-47
                nc.scalar.activation(
                    dst[:, :, 30:31],
                    tp[:, j:j + 1, 960 + gi * HID:960 + (gi + 1) * HID]
                    .rearrange("p a c -> p c a"), fn)
            g1 = sc_pool.tile([128, HID, D], F16, tag="g1")
            g2 = sc_pool.tile([128, HID, D], F16, tag="g2")
            nc.vector.scalar_tensor_tensor(
                g1[:], f1[:], 1.0, zt[:], ALU.subtract, ALU.mult)
            nc.vector.scalar_tensor_tensor(
                g2[:], f2[:], 1.0, zt[:, :, ::-1], ALU.subtract, ALU.mult)
            nc.vector.memset(f1[:, :, 0:1], 0.0)
            nc.vector.memset(f2[:, :, 0:1], 0.0)
            h1 = sc_pool.tile([128, HID, D], F32, tag="h1")
            h2 = sc_pool.tile([128, HID, D], F32, tag="h2")
            nc.vector.tensor_tensor_scan(
                h1[:].rearrange("p c d -> p (c d)"),
                f1[:].rearrange("p c d -> p (c d)"),
                g1[:].rearrange("p c d -> p (c d)"),
                0.0, ALU.mult, ALU.subtract)
            nc.vector.tensor_tensor_scan(
                h2[:].rearrange("p c d -> p (c d)"),
                f2[:].rearrange("p c d -> p (c d)"),
                g2[:].rearrange("p c d -> p (c d)"),
                0.0, ALU.mult, ALU.subtract)
            o = sc_pool.tile([128, HID, D], F32, tag="o")
            nc.vector.tensor_add(o[:], h1[:], h2[:, :, ::-1])
            nc.gpsimd.dma_start(out[c0:c0 + CHUNK], o[:])

        tix = 0
        for _rep in range(reps):
            for b_i in range(B):
                for hb_i in range(n_hblk):
                    xb = xbufs[tix % 2]
                    tix += 1
                    h0 = hb_i * HB
                    for kd in range(3):
                        nc.sync.dma_start(
                            xb[kd * 16:kd * 16 + 16].rearrange(
                                "p d t w -> p d (t w)"),
                            x_dram[:, kd:kd + D, b_i, h0:h0 + 2, :].rearrange(
                                "p d t w -> p d (t w)"))
                        nc.sync.dma_start(
                            xb[64 + kd * 16:64 + kd * 16 + 16].rearrange(
                                "p d t w -> p d (t w)"),
                            x_dram[:, kd:kd + D, b_i,
                                   h0 + 2:h0 + 4, :].rearrange(
                                "p d t w -> p d (t w)"))
                    s0 = b_i * (HSH * W) + h0 * W
                    g = g_pool.tile([96, 32, W], F16, tag="g")
                    for dc in range(n_dc if do_conv else 0):
                        d0 = dc * DC
                        dn = min(DC, D - d0)
                        ps = ps_pool.tile([96, DC * W], F32, tag="ps")
                        psv = ps[:, 0:dn * W].rearrange(
                            "p (d w) -> p d w", w=W)
                        k = 0
                        for p in range(2):
                            for kw in range(3):
                                nc.tensor.matmul(
                                    psv,
                                    wsb[0:112, k * 96:(k + 1) * 96],
                                    xb[0:112, d0:d0 + dn, p, kw:kw + W],
                                    start=(k == 0), stop=(k == NST - 1))
                                k += 1
                        if dc == n_dc - 1:
                            # j-swap extra: d=30 gates for j=1 at rows 0-47
                            k = 0
                            for p in range(2):
                                for kw in range(3):
                                    nc.tensor.matmul(
                                        ps[0:48, W:2 * W],
                                        wsb[0:112,
                                            576 + k * 48:576 + (k + 1) * 48],
                                        xb[0:112, d0, p, kw:kw + W],
                                        start=(k == 0), stop=(k == NST - 1))
                                    k += 1
                        gv = g[:, d0:d0 + dn, :].rearrange("p d w -> p (d w)")
                        if dc % 2 == 0:
                            nc.scalar.copy(gv, ps[:, 0:dn * W])
                        else:
                            nc.vector.tensor_copy(gv, ps[:, 0:dn * W])
                        if dc == n_dc - 1:
                            nc.scalar.copy(
                                g[0:48, 31, :], ps[0:48, W:2 * W])
                    if not (do_tp and do_conv):
                        continue
                    for wh in range(2):
                        tp = tp_pool.tile([128, 3, 1024], F16, tag="tp")
                        w0 = wh * 128
                        for d in range(30):
                            nc.tensor.transpose(
                                tp[:, d // 10,
                                   (d % 10) * 96:(d % 10) * 96 + 96],
                                g[:, d, w0:w0 + 128],
                                idsb[0:96, 0:96])
                        for j in range(2):
                            nc.tensor.transpose(
                                tp[:, j, 960:1008],
                                g[0:48, 30 + j, w0:w0 + 128],
                                idsb[0:48, 0:48])
                        if do_scan:
                            for j in range(2):
                                scan_chunk(tp, j, s0 + j * W + w0)

    nc.finalize()
    return nc


def _host_inputs(x, Wc, b):
    """x: [B, CIN, D, H, W] f32 full input. Returns list of 8 in_maps."""
    bf = np.float16
    # 6 stationaries: idx = t*3+kw, each [128, 96] with cols (j*48+co).
    # x tile: block A (rows 0-47) holds tile-rows (h0, h0+1) at t=0,1;
    # block B (rows 64-111) holds (h0+2, h0+3). Pass t streams A at row
    # h0+t and B at h0+2+t, so taps: A: kh = t - j, B: kh = 2 + t - j.
    wt = np.zeros((NST, 128, 2 * CO), np.float32)
    for t in range(2):
        for kw in range(3):
            idx = t * 3 + kw
            for j in range(2):
                c0 = j * CO
                for blk, khv in ((0, t - j), (64, 2 + t - j)):
                    if khv < 0 or khv > 2:
                        continue
                    for kd in range(3):
                        p0 = blk + kd * 16
                        wt[idx, p0:p0 + 16, c0:c0 + CO] = \
                            Wc[:, :, kd, khv, kw].T
    wt[0, 48, 0:CO] = b
    wt[0, 48, CO:2 * CO] = b
    # 6 j-swap stationaries (for d=30 j=1 gates at rows 0-47): the j=1
    # column block of the main set, as its own M=48 stationary.
    wt2 = np.zeros((NST, 128, CO), np.float32)
    for t in range(2):
        for kw in range(3):
            idx = t * 3 + kw
            for blk, khv in ((0, t - 1), (64, t + 1)):
                if khv < 0 or khv > 2:
                    continue
                for kd in range(3):
                    p0 = blk + kd * 16
                    wt2[idx, p0:p0 + 16, :] = Wc[:, :, kd, khv, kw].T
    wt2[0, 48, :] = b
    wts = np.concatenate(
        [wt.transpose(1, 0, 2).reshape(128, NST * 2 * CO),
         wt2.transpose(1, 0, 2).reshape(128, NST * CO)],
        axis=1).astype(bf)
    assert wts.shape == (128, WCOLS)
    auxa = np.zeros((16, FX), np.float32)
    auxa[0, :] = 1.0
    auxa = auxa.astype(bf)
    idn = np.eye(96, dtype=bf)

    xt = np.ascontiguousarray(x.transpose(1, 2, 0, 3, 4))  # [CIN,D,B,H,W]
    in_maps = []
    for c in range(N_CORES):
        hs, he = c * HSH, (c + 1) * HSH
        xp = np.zeros((CIN, D + 2, B, HSH + 2, WP), np.float32)
        lo = max(hs - 1, 0)
        hi = min(he + 1, H)
        xp[:, 1:D + 1, :, (lo - (hs - 1)):(hi - (hs - 1)), 1:W + 1] = \
            xt[:, :, :, lo:hi, :]
        in_maps.append({"x": xp.astype(bf), "wts": wts, "aux": auxa,
                        "idn": idn})
    return in_maps


_PROGRAM = None


def _get_program():
    global _PROGRAM
    if _PROGRAM is None:
        _PROGRAM = _build_program()
    return _PROGRAM


def run_sharded(in_maps, trace=False, **kw):
    from concourse import bass_utils
    nc = _get_program()
    return bass_utils.run_bass_kernel_spmd(
        nc, in_maps, core_ids=list(range(N_CORES)), trace=trace, **kw)


def _assemble(results):
    outf = np.empty((B, HID, D, H, W), np.float32)
    for c in range(N_CORES):
        raw = np.asarray(results[c]["out"])  # [S, HID, D]
        o = raw.reshape(B, HSH, W, HID, D).transpose(0, 3, 4, 1, 2)
        outf[:, :, :, c * HSH:(c + 1) * HSH, :] = o
    return outf


def kernel(x, W, b):
    x = np.asarray(x, np.float32)
    W = np.asarray(W, np.float32)
    b = np.asarray(b, np.float32)
    in_maps = _host_inputs(x, W, b)
    res = run_sharded(in_maps)
    return _assemble(res.results)
